# revision 13
# baseline (speedup 1.0000x reference)
"""Trainium2 Bass kernel for nn_Encoder_45870250721282.

Contract: kernel(**inputs) takes the FULL unsharded inputs (numpy) and
returns the FULL output tuple, matching reference.reference():

    (t_all[:, :, of], q_all[None][:, of], vertices, texture_map, tdiff, qdiff)

Strategy (8 NeuronCores, SPMD one program):
  - texture_map (1,16,2048,2048) f32 = 256 MiB is a pure passthrough ->
    flat-shard 8 ways; each core DRAM->DRAM copies its 32 MiB shard.
    This dominates: per-core HBM traffic ~32r+32w MiB -> memory roofline.
  - vertices normalize (mesh_normalize(ivertices+vertices_p), 1.2 MB) and
    the per-frame quaternion/translation math (240 frames) are replicated
    on every core (SPMD same program); host takes core 0's result. Their
    compute hides entirely under the texture DMA shadow.
"""

import numpy as np

import concourse.bass as bass
import concourse.bacc as bacc
import concourse.mybir as mybir
from concourse import bass_isa, tile
from concourse.bass_utils import run_bass_kernel_spmd
from concourse.tile import add_dep_helper

F32 = mybir.dt.float32
AX = mybir.AxisListType
OP = mybir.AluOpType

F = 240              # frames
V = 100000           # vertices
TEX_ELEMS = 16 * 2048 * 2048   # 67108864
NCORES = 8
TEX_SHARD = TEX_ELEMS // NCORES  # 8388608 elems = 32 MiB
TEX_CHUNKS = 4

VP = 25              # vertex partitions (few partitions -> one fat 48 KiB
VC = (V * 3) // VP   # descriptor per partition, so the SBUF load drains in
                     # a few round-robin turns against the texture packets)
NV = VC // 3         # vertices per partition

# small_in layout (1, 3360): q(960) | offq(960) | tr(720) | offt(720)
SI_Q, SI_OQ, SI_TR, SI_OT = 0, 960, 1920, 2640
SI_LEN = 3360
# small_out layout (1, 2160): q_all(960) | t_all(720) | tdiff(240) | qdiff(240)
SO_Q, SO_T, SO_TD, SO_QD = 0, 960, 1680, 1920
SO_LEN = 2160

_CACHE = {}


def _build():
    """Build + compile the SPMD program (identical on all 8 cores)."""
    nc = bacc.Bacc("TRN2", target_bir_lowering=False, debug=False)

    tex_in = nc.dram_tensor("tex_in", [TEX_SHARD], F32, kind="ExternalInput")
    tex_out = nc.dram_tensor("tex_out", [TEX_SHARD], F32, kind="ExternalOutput")
    verts_in = nc.dram_tensor("verts_in", [VP, 2 * VC], F32, kind="ExternalInput")
    small_in = nc.dram_tensor("small_in", [1, SI_LEN], F32, kind="ExternalInput")
    v_out = nc.dram_tensor("v_out", [VP, VC], F32, kind="ExternalOutput")
    small_out = nc.dram_tensor("small_out", [1, SO_LEN], F32, kind="ExternalOutput")

    with tile.TileContext(nc) as tc:
        with tc.tile_pool(name="p", bufs=1) as pool:
            # Small/vertex DMAs ride the ACT HWDGE ring (nc.scalar); with one
            # fat descriptor per partition they drain within the first few
            # engine round-robin turns against the texture packets, so the
            # dependent compute chain finishes far inside the copy's shadow.
            vt = pool.tile([VP, 2 * VC], F32)   # [iv | vp] packed per row
            nc.scalar.dma_start(out=vt[:, 0:VC], in_=verts_in[:, 0:VC])
            nc.scalar.dma_start(out=vt[:, VC:2 * VC], in_=verts_in[:, VC:2 * VC])
            sm = pool.tile([1, SI_LEN], F32)
            nc.scalar.dma_start(out=sm[:], in_=small_in[:])

            # ---------- texture passthrough: DRAM->DRAM, chunked ----------
            csz = TEX_SHARD // TEX_CHUNKS
            for i in range(TEX_CHUNKS):
                nc.sync.dma_start(
                    out=tex_out[i * csz:(i + 1) * csz],
                    in_=tex_in[i * csz:(i + 1) * csz],
                )

            # ---------- vertices: mesh_normalize(iv + vp) ----------
            # in-place into the iv half (SBUF per-partition budget is tight
            # with few fat partitions)
            v = vt[:, 0:VC]
            nc.vector.tensor_add(v, vt[:, 0:VC], vt[:, VC:2 * VC])

            v3 = v.rearrange("p (n c) -> p c n", c=3)         # (VP,3,NV)
            pmax = pool.tile([VP, 3], F32)
            pminN = pool.tile([VP, 3], F32)
            nc.vector.tensor_reduce(out=pmax[:], in_=v3, axis=AX.X, op=OP.max)
            nc.vector.tensor_reduce(out=pminN[:], in_=v3, axis=AX.X, op=OP.min)
            nc.vector.tensor_scalar_mul(pminN[:], pminN[:], -1.0)

            gmax = pool.tile([VP, 3], F32)    # global max per coord, all partitions
            gminN = pool.tile([VP, 3], F32)   # global -min per coord
            nc.gpsimd.partition_all_reduce(
                gmax[:], pmax[:], channels=VP, reduce_op=bass_isa.ReduceOp.max)
            nc.gpsimd.partition_all_reduce(
                gminN[:], pminN[:], channels=VP, reduce_op=bass_isa.ReduceOp.max)

            ctr = pool.tile([VP, 3], F32)     # (max+min)/2
            rh2 = pool.tile([VP, 3], F32)     # max-min = 2*big_per_coord
            nc.vector.tensor_sub(ctr[:], gmax[:], gminN[:])
            nc.vector.tensor_scalar_mul(ctr[:], ctr[:], 0.5)
            nc.vector.tensor_add(rh2[:], gmax[:], gminN[:])
            big2 = pool.tile([VP, 1], F32)
            nc.vector.tensor_reduce(out=big2[:], in_=rh2[:], axis=AX.X, op=OP.max)
            ib = pool.tile([VP, 1], F32)      # 1/big = 2/big2
            nc.vector.reciprocal(ib[:], big2[:])
            nc.vector.tensor_scalar_mul(ib[:], ib[:], 2.0)

            for c in range(3):
                nc.vector.tensor_scalar(
                    out=v3[:, c:c + 1, :],
                    in0=v3[:, c:c + 1, :],
                    scalar1=ctr[:, c:c + 1],
                    scalar2=ib[:, 0:1],
                    op0=OP.subtract,
                    op1=OP.mult,
                )
            nc.scalar.dma_start(out=v_out[:], in_=v)

            # ---------- per-frame quaternion / translation ----------
            so = pool.tile([1, SO_LEN], F32)

            def norm4(src_off):
                """Return tile holding src / ||src|| rows (1, 960)."""
                sq = pool.tile([1, F * 4], F32, tag=f"sq{src_off}")
                src = sm[:, src_off:src_off + F * 4]
                nc.vector.tensor_mul(sq[:], src, src)
                ss = pool.tile([1, F], F32, tag=f"ss{src_off}")
                nc.vector.tensor_reduce(
                    out=ss[:], in_=sq[:].rearrange("a (f c) -> a f c", c=4),
                    axis=AX.X, op=OP.add)
                nrm = pool.tile([1, F], F32, tag=f"nrm{src_off}")
                nc.scalar.sqrt(nrm[:], ss[:])
                rinv = pool.tile([1, F], F32, tag=f"rinv{src_off}")
                nc.vector.reciprocal(rinv[:], nrm[:])
                out = pool.tile([1, F * 4], F32, tag=f"nq{src_off}")
                rb = rinv[:].unsqueeze(2).broadcast_to([1, F, 4])
                nc.vector.tensor_mul(
                    out[:].rearrange("a (f c) -> a f c", c=4),
                    sm[:, src_off:src_off + F * 4].rearrange("a (f c) -> a f c", c=4),
                    rb)
                return out

            qn = norm4(SI_Q)       # normalized quaternion_p  (1, 960)
            offn = norm4(SI_OQ)    # normalized offset quats  (1, 960)

            def comp(t, c, base=0, n=4, cnt=F):
                # (1,1,cnt) strided view of component c
                return t[:, base:base + cnt * n].rearrange(
                    "a (f c) -> a c f", c=n)[:, c:c + 1, :]

            # Hamilton product qprod = qn * offn -> so[q_all region]
            # table: out[c] = sum of sign * qn[i] * offn[j]
            HAM = [
                [(+1, 0, 0), (-1, 1, 1), (-1, 2, 2), (-1, 3, 3)],
                [(+1, 1, 0), (+1, 2, 3), (-1, 3, 2), (+1, 0, 1)],
                [(+1, 2, 0), (+1, 3, 1), (-1, 1, 3), (+1, 0, 2)],
                [(+1, 1, 2), (-1, 2, 1), (+1, 3, 0), (+1, 0, 3)],
            ]
            t1 = pool.tile([1, F], F32)
            acc = pool.tile([1, F], F32)
            for c, terms in enumerate(HAM):
                s0, i0, j0 = terms[0]
                assert s0 == +1
                nc.vector.tensor_mul(
                    acc[:].unsqueeze(1), comp(qn, i0), comp(offn, j0))
                for (s, i, j) in terms[1:]:
                    nc.vector.tensor_mul(
                        t1[:].unsqueeze(1), comp(qn, i), comp(offn, j))
                    if s > 0:
                        nc.vector.tensor_add(acc[:], acc[:], t1[:])
                    else:
                        nc.vector.tensor_sub(acc[:], acc[:], t1[:])
                nc.vector.tensor_copy(comp(so, c, base=SO_Q), acc[:].unsqueeze(1))
            # frame 0 uses the bare normalized quaternion
            nc.vector.tensor_copy(so[:, SO_Q:SO_Q + 4], qn[:, 0:4])

            # t_all = tr + off_t, frame 0 = tr alone
            nc.vector.tensor_add(
                so[:, SO_T:SO_T + F * 3],
                sm[:, SI_TR:SI_TR + F * 3],
                sm[:, SI_OT:SI_OT + F * 3])
            nc.vector.tensor_copy(so[:, SO_T:SO_T + 3], sm[:, SI_TR:SI_TR + 3])

            # tdiff (opt_frames == arange): d = t[k]-t[k-1]; zero |d|<0.2 elems;
            # tdiff[k] = ||d||_2, tdiff[0] = 0
            d = pool.tile([1, (F - 1) * 3], F32)
            nc.vector.tensor_sub(
                d[:], so[:, SO_T + 3:SO_T + F * 3], so[:, SO_T:SO_T + (F - 1) * 3])
            dsq = pool.tile([1, (F - 1) * 3], F32)
            nc.vector.tensor_mul(dsq[:], d[:], d[:])
            # (d^2 >= 0.04) * d^2  == keep only |d| >= 0.2
            dm = pool.tile([1, (F - 1) * 3], F32)
            nc.vector.scalar_tensor_tensor(
                out=dm[:], in0=dsq[:], scalar=0.04, in1=dsq[:],
                op0=OP.is_ge, op1=OP.mult)
            tss = pool.tile([1, F - 1], F32)
            nc.vector.tensor_reduce(
                out=tss[:], in_=dm[:].rearrange("a (f c) -> a f c", c=3),
                axis=AX.X, op=OP.add)
            nc.scalar.sqrt(so[:, SO_TD + 1:SO_TD + F], tss[:])
            nc.vector.memset(so[:, SO_TD:SO_TD + 1], 0.0)

            # qdiff[k] = 1 - dot(q_all[k-1], q_all[k])^2, qdiff[0] = 0
            prod = pool.tile([1, (F - 1) * 4], F32)
            nc.vector.tensor_mul(
                prod[:], so[:, SO_Q:SO_Q + (F - 1) * 4], so[:, SO_Q + 4:SO_Q + F * 4])
            dots = pool.tile([1, F - 1], F32)
            nc.vector.tensor_reduce(
                out=dots[:], in_=prod[:].rearrange("a (f c) -> a f c", c=4),
                axis=AX.X, op=OP.add)
            dots2 = pool.tile([1, F - 1], F32)
            nc.vector.tensor_mul(dots2[:], dots[:], dots[:])
            nc.vector.tensor_scalar(
                out=so[:, SO_QD + 1:SO_QD + F], in0=dots2[:],
                scalar1=-1.0, scalar2=1.0, op0=OP.mult, op1=OP.add)
            nc.vector.memset(so[:, SO_QD:SO_QD + 1], 0.0)

            nc.scalar.dma_start(out=small_out[:], in_=so[:])

    nc.compile()
    return nc


def _get_nc():
    if "nc" not in _CACHE:
        _CACHE["nc"] = _build()
    return _CACHE["nc"]


def _make_in_maps(translation_p, quaternion_p, vertices_p, texture_map,
                  ivertices, offsets):
    tex_flat = np.ascontiguousarray(texture_map, dtype=np.float32).reshape(-1)
    iv2 = np.ascontiguousarray(ivertices, dtype=np.float32).reshape(VP, VC)
    vp2 = np.ascontiguousarray(vertices_p, dtype=np.float32).reshape(VP, VC)
    verts = np.concatenate([iv2, vp2], axis=1)  # (VP, 2*VC): [iv | vp] per row
    small = np.concatenate([
        np.asarray(quaternion_p, dtype=np.float32)[0].reshape(-1),       # q
        np.asarray(offsets, dtype=np.float32)[0, 0, :, 3:7].reshape(-1),  # offq
        np.asarray(translation_p, dtype=np.float32)[0, 0].reshape(-1),    # tr
        np.asarray(offsets, dtype=np.float32)[0, 0, :, 0:3].reshape(-1),  # offt
    ])[None].astype(np.float32)
    assert small.shape == (1, SI_LEN)
    in_maps = []
    for i in range(NCORES):
        in_maps.append({
            "tex_in": tex_flat[i * TEX_SHARD:(i + 1) * TEX_SHARD],
            "verts_in": verts,
            "small_in": small,
        })
    return in_maps


def _host_small_outputs(t_all, q_all, of):
    """General (non-arange opt_frames) fallback for the tiny outputs,
    replicating the reference math in numpy f32."""
    of = np.asarray(of)
    w = (of - np.concatenate([of[:1], of[:-1]])).astype(np.float32)
    t = t_all[of]                                   # (K,3)
    vd = np.abs(t[1:] - t[:-1])
    vd = np.where(vd < 0.2, np.zeros_like(vd), vd)
    vd = np.concatenate([np.zeros_like(vd[:1]), vd], axis=0)
    tdiff = w * np.linalg.norm(vd, axis=1)
    qlast = q_all[-1]
    d0 = np.float32(1.0) - np.sum(qlast * qlast) ** 2
    qa = q_all[of[1:] - 1]
    qb = q_all[of[1:]]
    kd = 1.0 - np.sum(qa * qb, axis=-1) ** 2
    qdiff = w * np.concatenate([d0[None], kd], axis=0)
    return tdiff.astype(np.float32), qdiff.astype(np.float32)


def _run(inputs, trace=False, **spmd_kwargs):
    nc = _get_nc()
    in_maps = _make_in_maps(
        inputs["translation_p"], inputs["quaternion_p"], inputs["vertices_p"],
        inputs["texture_map"], inputs["ivertices"], inputs["offsets"])
    res = run_bass_kernel_spmd(
        nc, in_maps, core_ids=list(range(NCORES)), trace=trace, **spmd_kwargs)
    return res


def _assemble(results, opt_frames):
    tex = np.concatenate(
        [results[i]["tex_out"] for i in range(NCORES)]
    ).reshape(1, 16, 2048, 2048)
    vertices = results[0]["v_out"].reshape(1, V, 3)
    so = results[0]["small_out"][0]
    q_all = so[SO_Q:SO_Q + F * 4].reshape(F, 4)
    t_all = so[SO_T:SO_T + F * 3].reshape(F, 3)
    tdiff = so[SO_TD:SO_TD + F].copy()
    qdiff = so[SO_QD:SO_QD + F].copy()

    of = np.asarray(opt_frames)
    if not np.array_equal(of, np.arange(F, dtype=of.dtype)):
        tdiff, qdiff = _host_small_outputs(t_all, q_all, of)

    t_out = t_all[of][None, None]          # (1,1,K,3)
    q_out = q_all[of][None]                # (1,K,4)
    return (t_out, q_out, vertices, tex, tdiff, qdiff)


def kernel(translation_p, quaternion_p, vertices_p, texture_map,
           ivertices, offsets, opt_frames):
    res = _run(dict(
        translation_p=translation_p, quaternion_p=quaternion_p,
        vertices_p=vertices_p, texture_map=texture_map,
        ivertices=ivertices, offsets=offsets))
    return _assemble(res.results, opt_frames)


# revision 15
# speedup vs baseline: 1.1553x; 1.1553x over previous
"""Trainium2 Bass kernel for nn_Encoder_45870250721282.

Contract: kernel(**inputs) takes the FULL unsharded inputs (numpy) and
returns the FULL output tuple, matching reference.reference():

    (t_all[:, :, of], q_all[None][:, of], vertices, texture_map, tdiff, qdiff)

Strategy (8 NeuronCores, SPMD one program):
  - texture_map (1,16,2048,2048) f32 = 256 MiB is a pure passthrough ->
    flat-shard 8 ways; each core DRAM->DRAM copies its 32 MiB shard.
    This dominates: per-core HBM traffic ~32r+32w MiB -> memory roofline.
  - vertices normalize (mesh_normalize(ivertices+vertices_p), 1.2 MB) and
    the per-frame quaternion/translation math (240 frames) are replicated
    on every core (SPMD same program); host takes core 0's result. Their
    compute hides entirely under the texture DMA shadow.
"""

import numpy as np

import concourse.bass as bass
import concourse.bacc as bacc
import concourse.mybir as mybir
from concourse import bass_isa, tile
from concourse.bass_utils import run_bass_kernel_spmd
from concourse.tile import add_dep_helper

F32 = mybir.dt.float32
AX = mybir.AxisListType
OP = mybir.AluOpType

F = 240              # frames
V = 100000           # vertices
TEX_ELEMS = 16 * 2048 * 2048   # 67108864
NCORES = 8
TEX_SHARD = TEX_ELEMS // NCORES  # 8388608 elems = 32 MiB
TEX_CHUNKS = 4

VP = 125             # vertex partitions (wide -> all 16 DMA engines + fast DVE)
VC = (V * 3) // VP   # 2400 cols/partition (800 vertices)

# small_in layout (1, 3360): q(960) | offq(960) | tr(720) | offt(720)
SI_Q, SI_OQ, SI_TR, SI_OT = 0, 960, 1920, 2640
SI_LEN = 3360
# small_out layout (1, 2160): q_all(960) | t_all(720) | tdiff(240) | qdiff(240)
SO_Q, SO_T, SO_TD, SO_QD = 0, 960, 1680, 1920
SO_LEN = 2160

_CACHE = {}


def _build():
    """Build + compile the SPMD program (identical on all 8 cores)."""
    nc = bacc.Bacc("TRN2", target_bir_lowering=False, debug=False)

    tex_in = nc.dram_tensor("tex_in", [TEX_SHARD], F32, kind="ExternalInput")
    tex_out = nc.dram_tensor("tex_out", [TEX_SHARD], F32, kind="ExternalOutput")
    verts_in = nc.dram_tensor("verts_in", [VP, 2 * VC], F32, kind="ExternalInput")
    small_in = nc.dram_tensor("small_in", [1, SI_LEN], F32, kind="ExternalInput")
    v_out = nc.dram_tensor("v_out", [VP, VC], F32, kind="ExternalOutput")
    small_out = nc.dram_tensor("small_out", [1, SO_LEN], F32, kind="ExternalOutput")

    with tile.TileContext(nc) as tc:
        with tc.tile_pool(name="p", bufs=1) as pool:
            # Queueing plan (engines round-robin both HWDGE rings at packet
            # granularity, FIFO within a ring):
            #   sync ring:   verts load FIRST, then tex chunks 0..2 — the
            #                FIFO order acts as a gate without a semaphore.
            #   scalar ring: small load + tex chunk 3 up front, later the
            #                vout/small_out writes (issued once DVE is done,
            #                drain while the sync ring still streams tex).
            vt = pool.tile([VP, 2 * VC], F32)   # [iv | vp] packed per row
            nc.sync.dma_start(out=vt[:], in_=verts_in[:])
            sm = pool.tile([1, SI_LEN], F32)
            nc.scalar.dma_start(out=sm[:], in_=small_in[:])

            # ---------- texture passthrough: DRAM->DRAM, chunked ----------
            csz = TEX_SHARD // TEX_CHUNKS
            for i in range(TEX_CHUNKS):
                eng = nc.scalar if i == TEX_CHUNKS - 1 else nc.sync
                eng.dma_start(
                    out=tex_out[i * csz:(i + 1) * csz],
                    in_=tex_in[i * csz:(i + 1) * csz],
                )

            # ---------- vertices: mesh_normalize(iv + vp) ----------
            # in-place into the iv half (SBUF per-partition budget is tight
            # with few fat partitions)
            v = vt[:, 0:VC]
            nc.vector.tensor_add(v, vt[:, 0:VC], vt[:, VC:2 * VC])

            v3 = v.rearrange("p (n c) -> p c n", c=3)         # (VP,3,NV)
            pmax = pool.tile([VP, 3], F32)
            pminN = pool.tile([VP, 3], F32)
            nc.vector.tensor_reduce(out=pmax[:], in_=v3, axis=AX.X, op=OP.max)
            nc.vector.tensor_reduce(out=pminN[:], in_=v3, axis=AX.X, op=OP.min)
            nc.vector.tensor_scalar_mul(pminN[:], pminN[:], -1.0)

            gmax = pool.tile([VP, 3], F32)    # global max per coord, all partitions
            gminN = pool.tile([VP, 3], F32)   # global -min per coord
            nc.gpsimd.partition_all_reduce(
                gmax[:], pmax[:], channels=VP, reduce_op=bass_isa.ReduceOp.max)
            nc.gpsimd.partition_all_reduce(
                gminN[:], pminN[:], channels=VP, reduce_op=bass_isa.ReduceOp.max)

            ctr = pool.tile([VP, 3], F32)     # (max+min)/2
            rh2 = pool.tile([VP, 3], F32)     # max-min = 2*big_per_coord
            nc.vector.tensor_sub(ctr[:], gmax[:], gminN[:])
            nc.vector.tensor_scalar_mul(ctr[:], ctr[:], 0.5)
            nc.vector.tensor_add(rh2[:], gmax[:], gminN[:])
            big2 = pool.tile([VP, 1], F32)
            nc.vector.tensor_reduce(out=big2[:], in_=rh2[:], axis=AX.X, op=OP.max)
            ib = pool.tile([VP, 1], F32)      # 1/big = 2/big2
            nc.vector.reciprocal(ib[:], big2[:])
            nc.vector.tensor_scalar_mul(ib[:], ib[:], 2.0)

            for c in range(3):
                nc.vector.tensor_scalar(
                    out=v3[:, c:c + 1, :],
                    in0=v3[:, c:c + 1, :],
                    scalar1=ctr[:, c:c + 1],
                    scalar2=ib[:, 0:1],
                    op0=OP.subtract,
                    op1=OP.mult,
                )
            nc.scalar.dma_start(out=v_out[:], in_=v)

            # ---------- per-frame quaternion / translation ----------
            so = pool.tile([1, SO_LEN], F32)

            def norm4(src_off):
                """Return tile holding src / ||src|| rows (1, 960)."""
                sq = pool.tile([1, F * 4], F32, tag=f"sq{src_off}")
                src = sm[:, src_off:src_off + F * 4]
                nc.vector.tensor_mul(sq[:], src, src)
                ss = pool.tile([1, F], F32, tag=f"ss{src_off}")
                nc.vector.tensor_reduce(
                    out=ss[:], in_=sq[:].rearrange("a (f c) -> a f c", c=4),
                    axis=AX.X, op=OP.add)
                nrm = pool.tile([1, F], F32, tag=f"nrm{src_off}")
                nc.scalar.sqrt(nrm[:], ss[:])
                rinv = pool.tile([1, F], F32, tag=f"rinv{src_off}")
                nc.vector.reciprocal(rinv[:], nrm[:])
                out = pool.tile([1, F * 4], F32, tag=f"nq{src_off}")
                rb = rinv[:].unsqueeze(2).broadcast_to([1, F, 4])
                nc.vector.tensor_mul(
                    out[:].rearrange("a (f c) -> a f c", c=4),
                    sm[:, src_off:src_off + F * 4].rearrange("a (f c) -> a f c", c=4),
                    rb)
                return out

            qn = norm4(SI_Q)       # normalized quaternion_p  (1, 960)
            offn = norm4(SI_OQ)    # normalized offset quats  (1, 960)

            def comp(t, c, base=0, n=4, cnt=F):
                # (1,1,cnt) strided view of component c
                return t[:, base:base + cnt * n].rearrange(
                    "a (f c) -> a c f", c=n)[:, c:c + 1, :]

            # Hamilton product qprod = qn * offn -> so[q_all region]
            # table: out[c] = sum of sign * qn[i] * offn[j]
            HAM = [
                [(+1, 0, 0), (-1, 1, 1), (-1, 2, 2), (-1, 3, 3)],
                [(+1, 1, 0), (+1, 2, 3), (-1, 3, 2), (+1, 0, 1)],
                [(+1, 2, 0), (+1, 3, 1), (-1, 1, 3), (+1, 0, 2)],
                [(+1, 1, 2), (-1, 2, 1), (+1, 3, 0), (+1, 0, 3)],
            ]
            t1 = pool.tile([1, F], F32)
            acc = pool.tile([1, F], F32)
            for c, terms in enumerate(HAM):
                s0, i0, j0 = terms[0]
                assert s0 == +1
                nc.vector.tensor_mul(
                    acc[:].unsqueeze(1), comp(qn, i0), comp(offn, j0))
                for (s, i, j) in terms[1:]:
                    nc.vector.tensor_mul(
                        t1[:].unsqueeze(1), comp(qn, i), comp(offn, j))
                    if s > 0:
                        nc.vector.tensor_add(acc[:], acc[:], t1[:])
                    else:
                        nc.vector.tensor_sub(acc[:], acc[:], t1[:])
                nc.vector.tensor_copy(comp(so, c, base=SO_Q), acc[:].unsqueeze(1))
            # frame 0 uses the bare normalized quaternion
            nc.vector.tensor_copy(so[:, SO_Q:SO_Q + 4], qn[:, 0:4])

            # t_all = tr + off_t, frame 0 = tr alone
            nc.vector.tensor_add(
                so[:, SO_T:SO_T + F * 3],
                sm[:, SI_TR:SI_TR + F * 3],
                sm[:, SI_OT:SI_OT + F * 3])
            nc.vector.tensor_copy(so[:, SO_T:SO_T + 3], sm[:, SI_TR:SI_TR + 3])

            # tdiff (opt_frames == arange): d = t[k]-t[k-1]; zero |d|<0.2 elems;
            # tdiff[k] = ||d||_2, tdiff[0] = 0
            d = pool.tile([1, (F - 1) * 3], F32)
            nc.vector.tensor_sub(
                d[:], so[:, SO_T + 3:SO_T + F * 3], so[:, SO_T:SO_T + (F - 1) * 3])
            dsq = pool.tile([1, (F - 1) * 3], F32)
            nc.vector.tensor_mul(dsq[:], d[:], d[:])
            # (d^2 >= 0.04) * d^2  == keep only |d| >= 0.2
            dm = pool.tile([1, (F - 1) * 3], F32)
            nc.vector.scalar_tensor_tensor(
                out=dm[:], in0=dsq[:], scalar=0.04, in1=dsq[:],
                op0=OP.is_ge, op1=OP.mult)
            tss = pool.tile([1, F - 1], F32)
            nc.vector.tensor_reduce(
                out=tss[:], in_=dm[:].rearrange("a (f c) -> a f c", c=3),
                axis=AX.X, op=OP.add)
            nc.scalar.sqrt(so[:, SO_TD + 1:SO_TD + F], tss[:])
            nc.vector.memset(so[:, SO_TD:SO_TD + 1], 0.0)

            # qdiff[k] = 1 - dot(q_all[k-1], q_all[k])^2, qdiff[0] = 0
            prod = pool.tile([1, (F - 1) * 4], F32)
            nc.vector.tensor_mul(
                prod[:], so[:, SO_Q:SO_Q + (F - 1) * 4], so[:, SO_Q + 4:SO_Q + F * 4])
            dots = pool.tile([1, F - 1], F32)
            nc.vector.tensor_reduce(
                out=dots[:], in_=prod[:].rearrange("a (f c) -> a f c", c=4),
                axis=AX.X, op=OP.add)
            dots2 = pool.tile([1, F - 1], F32)
            nc.vector.tensor_mul(dots2[:], dots[:], dots[:])
            nc.vector.tensor_scalar(
                out=so[:, SO_QD + 1:SO_QD + F], in0=dots2[:],
                scalar1=-1.0, scalar2=1.0, op0=OP.mult, op1=OP.add)
            nc.vector.memset(so[:, SO_QD:SO_QD + 1], 0.0)

            nc.scalar.dma_start(out=small_out[:], in_=so[:])

    nc.compile()
    return nc


def _get_nc():
    if "nc" not in _CACHE:
        _CACHE["nc"] = _build()
    return _CACHE["nc"]


def _make_in_maps(translation_p, quaternion_p, vertices_p, texture_map,
                  ivertices, offsets):
    tex_flat = np.ascontiguousarray(texture_map, dtype=np.float32).reshape(-1)
    iv2 = np.ascontiguousarray(ivertices, dtype=np.float32).reshape(VP, VC)
    vp2 = np.ascontiguousarray(vertices_p, dtype=np.float32).reshape(VP, VC)
    verts = np.concatenate([iv2, vp2], axis=1)  # (VP, 2*VC): [iv | vp] per row
    small = np.concatenate([
        np.asarray(quaternion_p, dtype=np.float32)[0].reshape(-1),       # q
        np.asarray(offsets, dtype=np.float32)[0, 0, :, 3:7].reshape(-1),  # offq
        np.asarray(translation_p, dtype=np.float32)[0, 0].reshape(-1),    # tr
        np.asarray(offsets, dtype=np.float32)[0, 0, :, 0:3].reshape(-1),  # offt
    ])[None].astype(np.float32)
    assert small.shape == (1, SI_LEN)
    in_maps = []
    for i in range(NCORES):
        in_maps.append({
            "tex_in": tex_flat[i * TEX_SHARD:(i + 1) * TEX_SHARD],
            "verts_in": verts,
            "small_in": small,
        })
    return in_maps


def _host_small_outputs(t_all, q_all, of):
    """General (non-arange opt_frames) fallback for the tiny outputs,
    replicating the reference math in numpy f32."""
    of = np.asarray(of)
    w = (of - np.concatenate([of[:1], of[:-1]])).astype(np.float32)
    t = t_all[of]                                   # (K,3)
    vd = np.abs(t[1:] - t[:-1])
    vd = np.where(vd < 0.2, np.zeros_like(vd), vd)
    vd = np.concatenate([np.zeros_like(vd[:1]), vd], axis=0)
    tdiff = w * np.linalg.norm(vd, axis=1)
    qlast = q_all[-1]
    d0 = np.float32(1.0) - np.sum(qlast * qlast) ** 2
    qa = q_all[of[1:] - 1]
    qb = q_all[of[1:]]
    kd = 1.0 - np.sum(qa * qb, axis=-1) ** 2
    qdiff = w * np.concatenate([d0[None], kd], axis=0)
    return tdiff.astype(np.float32), qdiff.astype(np.float32)


def _run(inputs, trace=False, **spmd_kwargs):
    nc = _get_nc()
    in_maps = _make_in_maps(
        inputs["translation_p"], inputs["quaternion_p"], inputs["vertices_p"],
        inputs["texture_map"], inputs["ivertices"], inputs["offsets"])
    res = run_bass_kernel_spmd(
        nc, in_maps, core_ids=list(range(NCORES)), trace=trace, **spmd_kwargs)
    return res


def _assemble(results, opt_frames):
    tex = np.concatenate(
        [results[i]["tex_out"] for i in range(NCORES)]
    ).reshape(1, 16, 2048, 2048)
    vertices = results[0]["v_out"].reshape(1, V, 3)
    so = results[0]["small_out"][0]
    q_all = so[SO_Q:SO_Q + F * 4].reshape(F, 4)
    t_all = so[SO_T:SO_T + F * 3].reshape(F, 3)
    tdiff = so[SO_TD:SO_TD + F].copy()
    qdiff = so[SO_QD:SO_QD + F].copy()

    of = np.asarray(opt_frames)
    if not np.array_equal(of, np.arange(F, dtype=of.dtype)):
        tdiff, qdiff = _host_small_outputs(t_all, q_all, of)

    t_out = t_all[of][None, None]          # (1,1,K,3)
    q_out = q_all[of][None]                # (1,K,4)
    return (t_out, q_out, vertices, tex, tdiff, qdiff)


def kernel(translation_p, quaternion_p, vertices_p, texture_map,
           ivertices, offsets, opt_frames):
    res = _run(dict(
        translation_p=translation_p, quaternion_p=quaternion_p,
        vertices_p=vertices_p, texture_map=texture_map,
        ivertices=ivertices, offsets=offsets))
    return _assemble(res.results, opt_frames)


# revision 23
# speedup vs baseline: 1.3016x; 1.1267x over previous
"""Trainium2 Bass kernel for nn_Encoder_45870250721282.

Contract: kernel(**inputs) takes the FULL unsharded inputs (numpy) and
returns the FULL output tuple, matching reference.reference():

    (t_all[:, :, of], q_all[None][:, of], vertices, texture_map, tdiff, qdiff)

Strategy (8 NeuronCores, SPMD one program):
  - texture_map (1,16,2048,2048) f32 = 256 MiB is a pure passthrough ->
    flat-shard 8 ways; each core DRAM->DRAM copies its 32 MiB shard.
    This dominates: per-core HBM traffic ~32r+32w MiB -> memory roofline.
  - vertices normalize (mesh_normalize(ivertices+vertices_p), 1.2 MB) and
    the per-frame quaternion/translation math (240 frames) are replicated
    on every core (SPMD same program); host takes core 0's result. Their
    compute hides entirely under the texture DMA shadow.
"""

import numpy as np

import concourse.bass as bass
import concourse.bacc as bacc
import concourse.mybir as mybir
from concourse import bass_isa, tile
from concourse.bass_utils import run_bass_kernel_spmd
from concourse.tile import add_dep_helper

F32 = mybir.dt.float32
AX = mybir.AxisListType
OP = mybir.AluOpType

F = 240              # frames
V = 100000           # vertices
TEX_ELEMS = 16 * 2048 * 2048   # 67108864
NCORES = 8
TEX_SHARD = TEX_ELEMS // NCORES  # 8388608 elems = 32 MiB
TEX_CHUNKS = 4

VP = 128             # vertex partitions (wide -> all 16 DMA engines + fast DVE)
NVP = 800            # vertices per partition (padded: 128*800 = 102400 slots)
VC = NVP * 3         # 2400 cols/partition
VPAD = VP * NVP      # 102400
VSH = V // NCORES    # 12500 own vertices per core
OWN_P = 16           # own shard lives in partitions [0:16) = 12800 slots

# small_in layout (1, 3360): q(960) | offq(960) | tr(720) | offt(720)
SI_Q, SI_OQ, SI_TR, SI_OT = 0, 960, 1920, 2640
SI_LEN = 3360
# small_out layout (1, 2160): q_all(960) | t_all(720) | tdiff(240) | qdiff(240)
SO_Q, SO_T, SO_TD, SO_QD = 0, 960, 1680, 1920
SO_LEN = 2160

_CACHE = {}


def _build():
    """Build + compile the SPMD program (identical on all 8 cores)."""
    nc = bacc.Bacc("TRN2", target_bir_lowering=False, debug=False)

    tex_in = nc.dram_tensor("tex_in", [TEX_SHARD], F32, kind="ExternalInput")
    tex_out = nc.dram_tensor("tex_out", [TEX_SHARD], F32, kind="ExternalOutput")
    verts_in = nc.dram_tensor("verts_in", [VP, 2 * VC], F32, kind="ExternalInput")
    small_in = nc.dram_tensor("small_in", [1, SI_LEN], F32, kind="ExternalInput")
    v_out = nc.dram_tensor("v_out", [OWN_P, VC], F32, kind="ExternalOutput")
    small_out = nc.dram_tensor("small_out", [1, SO_LEN], F32, kind="ExternalOutput")

    with tile.TileContext(nc) as tc:
        with tc.tile_pool(name="p", bufs=1) as pool:
            # Queueing plan: both HWDGE rings carry texture the whole time
            # (two active rings sustain ~380-415 GB/s vs ~320 single-ring);
            # rings are byte-balanced so they run dry together. The verts
            # load leads the sync ring; the writes at the end are tiny
            # (own-shard only, 16 fat descriptors) so the tail is cheap.
            vt = pool.tile([VP, 2 * VC], F32)   # [iv | vp] packed per row
            nc.sync.dma_start(out=vt[:], in_=verts_in[:])
            sm = pool.tile([1, SI_LEN], F32)
            nc.scalar.dma_start(out=sm[:], in_=small_in[:])

            # ---------- texture passthrough: DRAM->DRAM, chunked ----------
            # sync ring carries the verts load (2.46 MB), so it gets the
            # smaller texture span; each span split in two for pipelining.
            a = (TEX_SHARD - VP * 2 * VC) // 2          # sync-ring tex elems
            spans = [(0, a // 2, nc.sync), (a // 2, a, nc.sync),
                     (a, (a + TEX_SHARD) // 2, nc.scalar),
                     ((a + TEX_SHARD) // 2, TEX_SHARD, nc.scalar)]
            for lo, hi, eng in spans:
                eng.dma_start(out=tex_out[lo:hi], in_=tex_in[lo:hi])

            # ---------- vertices: mesh_normalize(iv + vp) ----------
            # in-place into the iv half (SBUF per-partition budget is tight
            # with few fat partitions)
            v = vt[:, 0:VC]
            nc.vector.tensor_add(v, vt[:, 0:VC], vt[:, VC:2 * VC])

            v3 = v.rearrange("p (n c) -> p c n", c=3)         # (VP,3,NV)
            pmax = pool.tile([VP, 3], F32)
            pminN = pool.tile([VP, 3], F32)
            nc.vector.tensor_reduce(out=pmax[:], in_=v3, axis=AX.X, op=OP.max)
            nc.vector.tensor_reduce(out=pminN[:], in_=v3, axis=AX.X, op=OP.min)
            nc.vector.tensor_scalar_mul(pminN[:], pminN[:], -1.0)

            gmax = pool.tile([VP, 3], F32)    # global max per coord, all partitions
            gminN = pool.tile([VP, 3], F32)   # global -min per coord
            nc.gpsimd.partition_all_reduce(
                gmax[:], pmax[:], channels=VP, reduce_op=bass_isa.ReduceOp.max)
            nc.gpsimd.partition_all_reduce(
                gminN[:], pminN[:], channels=VP, reduce_op=bass_isa.ReduceOp.max)

            ctr = pool.tile([VP, 3], F32)     # (max+min)/2
            rh2 = pool.tile([VP, 3], F32)     # max-min = 2*big_per_coord
            nc.vector.tensor_sub(ctr[:], gmax[:], gminN[:])
            nc.vector.tensor_scalar_mul(ctr[:], ctr[:], 0.5)
            nc.vector.tensor_add(rh2[:], gmax[:], gminN[:])
            big2 = pool.tile([VP, 1], F32)
            nc.vector.tensor_reduce(out=big2[:], in_=rh2[:], axis=AX.X, op=OP.max)
            ib = pool.tile([VP, 1], F32)      # 1/big = 2/big2
            nc.vector.reciprocal(ib[:], big2[:])
            nc.vector.tensor_scalar_mul(ib[:], ib[:], 2.0)

            # normalize + write only this core's own shard (partitions 0:16)
            vo3 = vt[0:OWN_P, 0:VC].rearrange("p (n c) -> p c n", c=3)
            for c in range(3):
                nc.vector.tensor_scalar(
                    out=vo3[:, c:c + 1, :],
                    in0=vo3[:, c:c + 1, :],
                    scalar1=ctr[0:OWN_P, c:c + 1],
                    scalar2=ib[0:OWN_P, 0:1],
                    op0=OP.subtract,
                    op1=OP.mult,
                )
            nc.scalar.dma_start(out=v_out[:], in_=vt[0:OWN_P, 0:VC])

            # ---------- per-frame quaternion / translation ----------
            so = pool.tile([1, SO_LEN], F32)

            def norm4(src_off):
                """Return tile holding src / ||src|| rows (1, 960)."""
                sq = pool.tile([1, F * 4], F32, tag=f"sq{src_off}")
                src = sm[:, src_off:src_off + F * 4]
                nc.vector.tensor_mul(sq[:], src, src)
                ss = pool.tile([1, F], F32, tag=f"ss{src_off}")
                nc.vector.tensor_reduce(
                    out=ss[:], in_=sq[:].rearrange("a (f c) -> a f c", c=4),
                    axis=AX.X, op=OP.add)
                nrm = pool.tile([1, F], F32, tag=f"nrm{src_off}")
                nc.scalar.sqrt(nrm[:], ss[:])
                rinv = pool.tile([1, F], F32, tag=f"rinv{src_off}")
                nc.vector.reciprocal(rinv[:], nrm[:])
                out = pool.tile([1, F * 4], F32, tag=f"nq{src_off}")
                rb = rinv[:].unsqueeze(2).broadcast_to([1, F, 4])
                nc.vector.tensor_mul(
                    out[:].rearrange("a (f c) -> a f c", c=4),
                    sm[:, src_off:src_off + F * 4].rearrange("a (f c) -> a f c", c=4),
                    rb)
                return out

            qn = norm4(SI_Q)       # normalized quaternion_p  (1, 960)
            offn = norm4(SI_OQ)    # normalized offset quats  (1, 960)

            def comp(t, c, base=0, n=4, cnt=F):
                # (1,1,cnt) strided view of component c
                return t[:, base:base + cnt * n].rearrange(
                    "a (f c) -> a c f", c=n)[:, c:c + 1, :]

            # Hamilton product qprod = qn * offn -> so[q_all region]
            # table: out[c] = sum of sign * qn[i] * offn[j]
            HAM = [
                [(+1, 0, 0), (-1, 1, 1), (-1, 2, 2), (-1, 3, 3)],
                [(+1, 1, 0), (+1, 2, 3), (-1, 3, 2), (+1, 0, 1)],
                [(+1, 2, 0), (+1, 3, 1), (-1, 1, 3), (+1, 0, 2)],
                [(+1, 1, 2), (-1, 2, 1), (+1, 3, 0), (+1, 0, 3)],
            ]
            t1 = pool.tile([1, F], F32)
            acc = pool.tile([1, F], F32)
            for c, terms in enumerate(HAM):
                s0, i0, j0 = terms[0]
                assert s0 == +1
                nc.vector.tensor_mul(
                    acc[:].unsqueeze(1), comp(qn, i0), comp(offn, j0))
                for (s, i, j) in terms[1:]:
                    nc.vector.tensor_mul(
                        t1[:].unsqueeze(1), comp(qn, i), comp(offn, j))
                    if s > 0:
                        nc.vector.tensor_add(acc[:], acc[:], t1[:])
                    else:
                        nc.vector.tensor_sub(acc[:], acc[:], t1[:])
                nc.vector.tensor_copy(comp(so, c, base=SO_Q), acc[:].unsqueeze(1))
            # frame 0 uses the bare normalized quaternion
            nc.vector.tensor_copy(so[:, SO_Q:SO_Q + 4], qn[:, 0:4])

            # t_all = tr + off_t, frame 0 = tr alone
            nc.vector.tensor_add(
                so[:, SO_T:SO_T + F * 3],
                sm[:, SI_TR:SI_TR + F * 3],
                sm[:, SI_OT:SI_OT + F * 3])
            nc.vector.tensor_copy(so[:, SO_T:SO_T + 3], sm[:, SI_TR:SI_TR + 3])

            # tdiff (opt_frames == arange): d = t[k]-t[k-1]; zero |d|<0.2 elems;
            # tdiff[k] = ||d||_2, tdiff[0] = 0
            d = pool.tile([1, (F - 1) * 3], F32)
            nc.vector.tensor_sub(
                d[:], so[:, SO_T + 3:SO_T + F * 3], so[:, SO_T:SO_T + (F - 1) * 3])
            dsq = pool.tile([1, (F - 1) * 3], F32)
            nc.vector.tensor_mul(dsq[:], d[:], d[:])
            # (d^2 >= 0.04) * d^2  == keep only |d| >= 0.2
            dm = pool.tile([1, (F - 1) * 3], F32)
            nc.vector.scalar_tensor_tensor(
                out=dm[:], in0=dsq[:], scalar=0.04, in1=dsq[:],
                op0=OP.is_ge, op1=OP.mult)
            tss = pool.tile([1, F - 1], F32)
            nc.vector.tensor_reduce(
                out=tss[:], in_=dm[:].rearrange("a (f c) -> a f c", c=3),
                axis=AX.X, op=OP.add)
            nc.scalar.sqrt(so[:, SO_TD + 1:SO_TD + F], tss[:])
            nc.vector.memset(so[:, SO_TD:SO_TD + 1], 0.0)

            # qdiff[k] = 1 - dot(q_all[k-1], q_all[k])^2, qdiff[0] = 0
            prod = pool.tile([1, (F - 1) * 4], F32)
            nc.vector.tensor_mul(
                prod[:], so[:, SO_Q:SO_Q + (F - 1) * 4], so[:, SO_Q + 4:SO_Q + F * 4])
            dots = pool.tile([1, F - 1], F32)
            nc.vector.tensor_reduce(
                out=dots[:], in_=prod[:].rearrange("a (f c) -> a f c", c=4),
                axis=AX.X, op=OP.add)
            dots2 = pool.tile([1, F - 1], F32)
            nc.vector.tensor_mul(dots2[:], dots[:], dots[:])
            nc.vector.tensor_scalar(
                out=so[:, SO_QD + 1:SO_QD + F], in0=dots2[:],
                scalar1=-1.0, scalar2=1.0, op0=OP.mult, op1=OP.add)
            nc.vector.memset(so[:, SO_QD:SO_QD + 1], 0.0)

            nc.scalar.dma_start(out=small_out[:], in_=so[:])

    nc.compile()
    return nc


def _get_nc():
    if "nc" not in _CACHE:
        _CACHE["nc"] = _build()
    return _CACHE["nc"]


def _make_in_maps(translation_p, quaternion_p, vertices_p, texture_map,
                  ivertices, offsets):
    tex_flat = np.ascontiguousarray(texture_map, dtype=np.float32).reshape(-1)
    iv2 = np.asarray(ivertices, dtype=np.float32).reshape(V, 3)
    vp2 = np.asarray(vertices_p, dtype=np.float32).reshape(V, 3)
    small = np.concatenate([
        np.asarray(quaternion_p, dtype=np.float32)[0].reshape(-1),       # q
        np.asarray(offsets, dtype=np.float32)[0, 0, :, 3:7].reshape(-1),  # offq
        np.asarray(translation_p, dtype=np.float32)[0, 0].reshape(-1),    # tr
        np.asarray(offsets, dtype=np.float32)[0, 0, :, 0:3].reshape(-1),  # offt
    ])[None].astype(np.float32)
    assert small.shape == (1, SI_LEN)
    in_maps = []
    allv = np.arange(V)
    for i in range(NCORES):
        # Permute + pad vertices per core: own 12500 vertices -> partitions
        # [0:16) (so the output write is 16 fat descriptors); the rest +
        # pads (copies of vertex 0 -- no effect on global max/min) fill the
        # remaining partitions, present only for the global reduce.
        own = allv[i * VSH:(i + 1) * VSH]
        rest = np.concatenate([allv[:i * VSH], allv[(i + 1) * VSH:]])
        idx = np.concatenate([
            own, np.zeros(OWN_P * NVP - VSH, dtype=np.int64),
            rest, np.zeros(VPAD - OWN_P * NVP - (V - VSH), dtype=np.int64),
        ])
        ivq = iv2[idx].reshape(VP, VC)
        vpq = vp2[idx].reshape(VP, VC)
        in_maps.append({
            "tex_in": tex_flat[i * TEX_SHARD:(i + 1) * TEX_SHARD],
            "verts_in": np.concatenate([ivq, vpq], axis=1),
            "small_in": small,
        })
    return in_maps


def _host_small_outputs(t_all, q_all, of):
    """General (non-arange opt_frames) fallback for the tiny outputs,
    replicating the reference math in numpy f32."""
    of = np.asarray(of)
    w = (of - np.concatenate([of[:1], of[:-1]])).astype(np.float32)
    t = t_all[of]                                   # (K,3)
    vd = np.abs(t[1:] - t[:-1])
    vd = np.where(vd < 0.2, np.zeros_like(vd), vd)
    vd = np.concatenate([np.zeros_like(vd[:1]), vd], axis=0)
    tdiff = w * np.linalg.norm(vd, axis=1)
    qlast = q_all[-1]
    d0 = np.float32(1.0) - np.sum(qlast * qlast) ** 2
    qa = q_all[of[1:] - 1]
    qb = q_all[of[1:]]
    kd = 1.0 - np.sum(qa * qb, axis=-1) ** 2
    qdiff = w * np.concatenate([d0[None], kd], axis=0)
    return tdiff.astype(np.float32), qdiff.astype(np.float32)


def _run(inputs, trace=False, **spmd_kwargs):
    nc = _get_nc()
    in_maps = _make_in_maps(
        inputs["translation_p"], inputs["quaternion_p"], inputs["vertices_p"],
        inputs["texture_map"], inputs["ivertices"], inputs["offsets"])
    res = run_bass_kernel_spmd(
        nc, in_maps, core_ids=list(range(NCORES)), trace=trace, **spmd_kwargs)
    return res


def _assemble(results, opt_frames):
    tex = np.concatenate(
        [results[i]["tex_out"] for i in range(NCORES)]
    ).reshape(1, 16, 2048, 2048)
    vertices = np.concatenate(
        [results[i]["v_out"].reshape(-1, 3)[:VSH] for i in range(NCORES)]
    ).reshape(1, V, 3)
    so = results[0]["small_out"][0]
    q_all = so[SO_Q:SO_Q + F * 4].reshape(F, 4)
    t_all = so[SO_T:SO_T + F * 3].reshape(F, 3)
    tdiff = so[SO_TD:SO_TD + F].copy()
    qdiff = so[SO_QD:SO_QD + F].copy()

    of = np.asarray(opt_frames)
    if not np.array_equal(of, np.arange(F, dtype=of.dtype)):
        tdiff, qdiff = _host_small_outputs(t_all, q_all, of)

    t_out = t_all[of][None, None]          # (1,1,K,3)
    q_out = q_all[of][None]                # (1,K,4)
    return (t_out, q_out, vertices, tex, tdiff, qdiff)


def kernel(translation_p, quaternion_p, vertices_p, texture_map,
           ivertices, offsets, opt_frames):
    res = _run(dict(
        translation_p=translation_p, quaternion_p=quaternion_p,
        vertices_p=vertices_p, texture_map=texture_map,
        ivertices=ivertices, offsets=offsets))
    return _assemble(res.results, opt_frames)
